# revision 20
# baseline (speedup 1.0000x reference)
"""KV-cache attention Bass kernel for Trainium2, 8 NeuronCores.

Sharding: batch (4) x query-half (2) -> 8 cores. Each core projects Q for its
1024 query rows, projects the full new K/V for its batch (duplicated across the
core pair), and runs softmax(Q K^T / 8) V over the 8192-row concatenated cache.

Layout strategy (everything kept in matmul-native layouts, no on-device
transposes):
  - scores are computed TRANSPOSED: S^T[t, s] with lhsT = K^T tile, rhs = Q^T.
  - softmax over t (partition dim) uses exp with a constant shift (exact:
    softmax is shift-invariant) and gets the denominator from an extra
    all-ones column appended to V, so P^T @ [V | 1] yields both the
    numerator rows and the denominator in one PSUM accumulation.
  - P^T is exactly the stationary operand layout the PV matmul needs, so no
    transposes are ever required.
All matmuls run in float32r (full-rate fp32 mode, ~1.5e-4 rms error).
"""
import sys
import numpy as np

if "/opt/trn_rl_repo" not in sys.path:
    sys.path.insert(0, "/opt/trn_rl_repo")

import concourse.bacc as bacc
import concourse.mybir as mybir
from concourse.tile import TileContext
from concourse.bass_utils import run_bass_kernel_spmd

B, S_NEW, S_CACHE, D = 4, 2048, 6144, 1024
S_KV = S_CACHE + S_NEW            # 8192
SQ = S_NEW // 2                   # 1024 query rows per core
N_CORES = 8
P = 128
ET = D // P                       # 8 feature tiles
DT = D // P                       # 8 contraction tiles
CHUNK = 512                       # kv rows per chunk
N_CHUNKS = S_KV // CHUNK          # 16 (12 cached + 4 new)
N_CACHED_CHUNKS = S_CACHE // CHUNK
TT4 = CHUNK // P                  # 4 t-ptiles per chunk
SCALE = 0.125                     # 1/sqrt(64)
SHIFT = -16.0                     # constant softmax shift (exact)

F32 = mybir.dt.float32
F32R = mybir.dt.float32r

_cache = {}


def _build():
    nc = bacc.Bacc("TRN2", target_bir_lowering=False, debug=False,
                   num_devices=N_CORES)
    ht = nc.dram_tensor("ht", [P, DT * S_NEW], F32R, kind="ExternalInput")
    wq = nc.dram_tensor("wq", [P, DT * D], F32R, kind="ExternalInput")
    wk = nc.dram_tensor("wk", [P, DT * D], F32R, kind="ExternalInput")
    wv = nc.dram_tensor("wv", [P, DT * D], F32R, kind="ExternalInput")
    kcT = nc.dram_tensor("kcT", [P, ET, S_CACHE], F32R, kind="ExternalInput")
    vc = nc.dram_tensor("vc", [P, S_CACHE // P, D], F32R, kind="ExternalInput")
    bq = nc.dram_tensor("bq", [P, ET], F32, kind="ExternalInput")
    bk = nc.dram_tensor("bk", [P, ET], F32, kind="ExternalInput")
    bv = nc.dram_tensor("bv", [P, D], F32, kind="ExternalInput")
    ident = nc.dram_tensor("ident", [P, P], F32, kind="ExternalInput")
    out = nc.dram_tensor("out", [SQ, D], F32, kind="ExternalOutput")

    # Which query half this core handles is baked in host-side: the host
    # rotates ht's t-columns so THIS core's 1024 query rows sit at columns
    # [0, SQ) of every d-tile. The rotation also permutes the new K/V row
    # order identically on both cores of a pair, which is harmless -
    # attention is permutation-invariant over kv rows.

    with TileContext(nc) as tc:
        with tc.tile_pool(name="big", bufs=1) as big, \
             tc.tile_pool(name="bias", bufs=1) as biasp, \
             tc.tile_pool(name="spsum", bufs=2, space="PSUM") as spsum, \
             tc.tile_pool(name="dpsum", bufs=2, space="PSUM") as dpsum, \
             tc.tile_pool(name="opsum", bufs=2, space="PSUM") as opsum, \
             tc.tile_pool(name="dram", bufs=1, space="DRAM") as dpool:

            nkT_d = dpool.tile([P, ET, S_NEW], F32R, name="nkT_d")
            nv_d = dpool.tile([P, S_NEW // P, D], F32R, name="nv_d")

            qT_sb = big.tile([P, ET * SQ], F32R, name="qT_sb")
            bq_sb = biasp.tile([P, ET], F32, name="bq_sb")
            bk_sb = biasp.tile([P, ET], F32, name="bk_sb")
            bv_sb = biasp.tile([P, D], F32, name="bv_sb")
            sh_sb = biasp.tile([P, 1], F32, name="sh_sb")
            nc.vector.memset(sh_sb[:], SHIFT)
            ones_sb = biasp.tile([P, 2], F32, name="ones_sb")
            nc.vector.memset(ones_sb[:], 1.0)
            onesr_sb = biasp.tile([P, 2], F32R, name="onesr_sb")
            nc.vector.tensor_copy(onesr_sb[:], ones_sb[:])
            id_sb = biasp.tile([P, P], F32, name="id_sb")
            nc.sync.dma_start(out=id_sb[:], in_=ident[:])

            nc.sync.dma_start(out=bq_sb[:], in_=bq[:])
            nc.sync.dma_start(out=bk_sb[:], in_=bk[:])
            nc.sync.dma_start(out=bv_sb[:], in_=bv[:])

            early_cm = tc.tile_pool(name="early", bufs=1)
            earlyp = early_cm.__enter__()
            kt0_sb = earlyp.tile([P, ET, CHUNK], F32R, name="kt0_sb")
            v0_sb = earlyp.tile([P, TT4, D], F32R, name="v0_sb")
            nc.sync.dma_start(out=kt0_sb[:], in_=kcT[:, :, 0:CHUNK])
            nc.sync.dma_start(out=v0_sb[:], in_=vc[:, 0:TT4, :])

            self_a = tc.tile_pool(name="abig", bufs=1)
            abig = self_a.__enter__()
            wpool_cm = tc.tile_pool(name="wpool", bufs=2)
            wpool = wpool_cm.__enter__()
            stage_cm = tc.tile_pool(name="stage", bufs=3)
            stagep = stage_cm.__enter__()

            wq_sb = wpool.tile([P, DT * D], F32R, name="w_sb", tag="w")
            ht_sb = abig.tile([P, DT * S_NEW], F32R, name="ht_sb")
            for dt in range(DT):
                nc.sync.dma_start(out=wq_sb[:, dt * D:(dt + 1) * D],
                                  in_=wq[:, dt * D:(dt + 1) * D])
                nc.sync.dma_start(out=ht_sb[:, dt * S_NEW:(dt + 1) * S_NEW],
                                  in_=ht[:, dt * S_NEW:(dt + 1) * S_NEW])
            wk_sb = wpool.tile([P, DT * D], F32R, name="w_sb2", tag="w")
            nc.sync.dma_start(out=wk_sb[:], in_=wk[:])
            wv_sb = wpool.tile([P, DT * D], F32R, name="w_sb3", tag="w")
            nc.sync.dma_start(out=wv_sb[:], in_=wv[:])

            # ---- Phase A1: Q^T projection (queries are ht cols [0, SQ)) ----
            for et in range(ET):
                for sc in range(SQ // 512):
                    ps = spsum.tile([P, 512], F32, name="ps_q", tag="sp")
                    for dt in range(DT):
                        nc.tensor.matmul(
                            ps[:],
                            wq_sb[:, dt * D + et * P:dt * D + (et + 1) * P],
                            ht_sb[:, dt * S_NEW + sc * 512:dt * S_NEW + (sc + 1) * 512],
                            start=(dt == 0), stop=(dt == DT - 1))
                    nc.scalar.activation(
                        qT_sb[:, et * SQ + sc * 512:et * SQ + (sc + 1) * 512],
                        ps[:], mybir.ActivationFunctionType.Identity,
                        bias=bq_sb[:, et:et + 1])

            # ---- Phase A2: new K^T -> DRAM scratch ----
            for et in range(ET):
                for sc in range(S_NEW // 512):
                    ps = spsum.tile([P, 512], F32, name="ps_k", tag="sp")
                    for dt in range(DT):
                        nc.tensor.matmul(
                            ps[:],
                            wk_sb[:, dt * D + et * P:dt * D + (et + 1) * P],
                            ht_sb[:, dt * S_NEW + sc * 512:dt * S_NEW + (sc + 1) * 512],
                            start=(dt == 0), stop=(dt == DT - 1))
                    st = stagep.tile([P, 512], F32R, name="st_k", tag="stage")
                    nc.scalar.activation(
                        st[:], ps[:], mybir.ActivationFunctionType.Identity,
                        bias=bk_sb[:, et:et + 1])
                    nc.scalar.dma_start(
                        out=nkT_d[:, et, sc * 512:(sc + 1) * 512], in_=st[:])

            # ---- Phase A3: new V -> DRAM scratch ----
            for tt in range(S_NEW // P):
                for ec in range(D // 512):
                    ps = spsum.tile([P, 512], F32, name="ps_v", tag="sp")
                    for dt in range(DT):
                        nc.tensor.matmul(
                            ps[:],
                            ht_sb[:, dt * S_NEW + tt * P:dt * S_NEW + (tt + 1) * P],
                            wv_sb[:, dt * D + ec * 512:dt * D + (ec + 1) * 512],
                            start=(dt == 0), stop=(dt == DT - 1))
                    st = stagep.tile([P, 512], F32R, name="st_v", tag="stage")
                    nc.vector.tensor_add(st[:], ps[:], bv_sb[:, ec * 512:(ec + 1) * 512])
                    nc.scalar.dma_start(out=nv_d[:, tt, ec * 512:(ec + 1) * 512],
                                        in_=st[:])

            # free phase-A SBUF (ht, weights, staging) for phase-B pools
            stage_cm.__exit__(None, None, None)
            wpool_cm.__exit__(None, None, None)
            self_a.__exit__(None, None, None)

            kpool_cm = tc.tile_pool(name="kpool", bufs=2)
            kpool = kpool_cm.__enter__()
            vpool_cm = tc.tile_pool(name="vpool", bufs=2)
            vpool = vpool_cm.__enter__()
            ptpool_cm = tc.tile_pool(name="ptpool", bufs=2)
            ptpool = ptpool_cm.__enter__()
            fin_cm = tc.tile_pool(name="fin", bufs=2)
            finp = fin_cm.__enter__()
            obig_cm = tc.tile_pool(name="obig", bufs=1)
            obig = obig_cm.__enter__()
            out_acc = obig.tile([P, SQ // P, D], F32, name="out_acc")
            dn_acc = obig.tile([2, SQ], F32, name="dn_acc")

            # ---- Phase B: attention over 16 kv chunks ----
            for c in range(N_CHUNKS):
                if c == 0:
                    kt_sb, v_sb = kt0_sb, v0_sb
                elif True:
                    kt_sb = kpool.tile([P, ET, CHUNK], F32R, name="kt_sb")
                    v_sb = vpool.tile([P, TT4, D], F32R, name="v_sb")
                if c == 0:
                    pass
                elif c < N_CACHED_CHUNKS:
                    nc.sync.dma_start(out=kt_sb[:],
                                      in_=kcT[:, :, c * CHUNK:(c + 1) * CHUNK])
                    nc.sync.dma_start(out=v_sb[:],
                                      in_=vc[:, c * TT4:(c + 1) * TT4, :])
                else:
                    cc = c - N_CACHED_CHUNKS
                    nc.sync.dma_start(out=kt_sb[:],
                                      in_=nkT_d[:, :, cc * CHUNK:(cc + 1) * CHUNK])
                    nc.sync.dma_start(out=v_sb[:],
                                      in_=nv_d[:, cc * TT4:(cc + 1) * TT4, :])

                for sb in range(SQ // 512):
                    pt = ptpool.tile([P, TT4, 512], F32R, name="pt")
                    for tt4 in range(TT4):
                        stp = spsum.tile([P, 512], F32, name="stp", tag="sp")
                        for et in range(ET):
                            nc.tensor.matmul(
                                stp[:],
                                kt_sb[:, et, tt4 * P:(tt4 + 1) * P],
                                qT_sb[:, et * SQ + sb * 512:et * SQ + (sb + 1) * 512],
                                start=(et == 0), stop=(et == ET - 1))
                        nc.scalar.activation(
                            pt[:, tt4, :], stp[:],
                            mybir.ActivationFunctionType.Exp,
                            bias=sh_sb[:], scale=SCALE)
                    dps = dpsum.tile([2, 512], F32, name="dps", tag="dps")
                    for tt4 in range(TT4):
                        nc.tensor.matmul(dps[:], onesr_sb[:],
                                         pt[:, tt4, :],
                                         start=(tt4 == 0), stop=(tt4 == TT4 - 1))
                    if c == 0:
                        nc.vector.tensor_copy(
                            dn_acc[0:2, sb * 512:(sb + 1) * 512], dps[0:2, :])
                    else:
                        nc.vector.tensor_add(
                            dn_acc[0:2, sb * 512:(sb + 1) * 512],
                            dn_acc[0:2, sb * 512:(sb + 1) * 512], dps[0:2, :])
                    for si in range(4):
                        si_g = sb * 4 + si
                        po = opsum.tile([P, D], F32, name="po")
                        for tt4 in range(TT4):
                            lhs = pt[:, tt4, si * P:(si + 1) * P]
                            st0 = (tt4 == 0)
                            sp1 = (tt4 == TT4 - 1)
                            nc.tensor.matmul(po[:, 0:512], lhs,
                                             v_sb[:, tt4, 0:512],
                                             start=st0, stop=sp1)
                            nc.tensor.matmul(po[:, 512:1024], lhs,
                                             v_sb[:, tt4, 512:1024],
                                             start=st0, stop=sp1)
                        if c == 0:
                            nc.vector.tensor_copy(out_acc[:, si_g, :], po[:])
                        else:
                            nc.vector.tensor_add(out_acc[:, si_g, :],
                                                 out_acc[:, si_g, :], po[:])

            # ---- Final: normalize and store ----
            for si_g in range(SQ // P):
                tps = dpsum.tile([P, 2], F32, name="tps", tag="dps")
                nc.tensor.matmul(
                    tps[:], dn_acc[0:2, si_g * P:(si_g + 1) * P],
                    id_sb[0:2, 0:2], start=True, stop=True)
                rec = finp.tile([P, 1], F32, name="rec")
                nc.vector.reciprocal(rec[:], tps[:, 0:1])
                ost = finp.tile([P, D], F32, name="ost")
                nc.scalar.activation(ost[:], out_acc[:, si_g, :D],
                                     mybir.ActivationFunctionType.Copy,
                                     scale=rec[:])
                nc.sync.dma_start(out=out[si_g * P:(si_g + 1) * P, :], in_=ost[:])

            obig_cm.__exit__(None, None, None)
            fin_cm.__exit__(None, None, None)
            ptpool_cm.__exit__(None, None, None)
            vpool_cm.__exit__(None, None, None)
            kpool_cm.__exit__(None, None, None)
            early_cm.__exit__(None, None, None)

    nc.compile()
    return nc


def _prep(hidden_states, cached_key, cached_value, Wq, bq, Wk, bk, Wv, bv):
    """Host-side resharding into SBUF-image layouts (pure reshapes/copies)."""
    def ptile_cols(a):  # [R, C] with R = n*128 -> [128, n*C] (partition-major)
        n = a.shape[0] // P
        return np.ascontiguousarray(
            a.reshape(n, P, a.shape[1]).transpose(1, 0, 2)).reshape(P, -1)

    w_h = {}
    for nm, W in (("wq", Wq), ("wk", Wk), ("wv", Wv)):
        w_h[nm] = ptile_cols(np.ascontiguousarray(W.T))          # [128, 8*1024]
    bq_h = np.ascontiguousarray(bq.reshape(ET, P).T)             # [128, 8]
    bk_h = np.ascontiguousarray(bk.reshape(ET, P).T)
    bv_h = np.ascontiguousarray(np.broadcast_to(bv, (P, D)))     # [128, 1024]
    id_h = np.eye(P, dtype=np.float32)

    in_maps = []
    for b in range(B):
        ht_full = ptile_cols(np.ascontiguousarray(hidden_states[b].T))  # [128, 8*2048]
        kcT_h = ptile_cols(np.ascontiguousarray(cached_key[b].T)) \
            .reshape(P, ET, S_CACHE)
        vc_h = np.ascontiguousarray(
            cached_value[b].reshape(S_CACHE // P, P, D).transpose(1, 0, 2))
        for h in range(2):
            # pack this core's query rows into ht cols [0, SQ) of each d-tile
            ht_v = ht_full.reshape(P, DT, S_NEW)
            if h == 0:
                ht_c = ht_full
            else:
                ht_c = np.ascontiguousarray(
                    np.concatenate([ht_v[:, :, SQ:], ht_v[:, :, :SQ]], axis=2)
                ).reshape(P, DT * S_NEW)
            in_maps.append({
                "ht": ht_c, "kcT": kcT_h, "vc": vc_h,
                "wq": w_h["wq"], "wk": w_h["wk"], "wv": w_h["wv"],
                "bq": bq_h, "bk": bk_h, "bv": bv_h, "ident": id_h,
                "hsel": np.zeros((1, 1), np.float32),
            })
    return in_maps


def kernel(hidden_states, cached_key, cached_value, Wq, bq, Wk, bk, Wv, bv,
           _trace=False):
    if "nc" not in _cache:
        _cache["nc"] = _build()
    nc = _cache["nc"]
    in_maps = _prep(
        np.asarray(hidden_states, dtype=np.float32),
        np.asarray(cached_key, dtype=np.float32),
        np.asarray(cached_value, dtype=np.float32),
        np.asarray(Wq, dtype=np.float32), np.asarray(bq, dtype=np.float32),
        np.asarray(Wk, dtype=np.float32), np.asarray(bk, dtype=np.float32),
        np.asarray(Wv, dtype=np.float32), np.asarray(bv, dtype=np.float32))
    res = run_bass_kernel_spmd(nc, in_maps, list(range(N_CORES)), trace=_trace)
    _cache["last_result"] = res
    out = np.empty((B, S_NEW, D), np.float32)
    for b in range(B):
        for h in range(2):
            out[b, h * SQ:(h + 1) * SQ, :] = res.results[2 * b + h]["out"]
    return out


# revision 21
# speedup vs baseline: 1.0270x; 1.0270x over previous
"""KV-cache attention Bass kernel for Trainium2, 8 NeuronCores.

Sharding: batch (4) x query-half (2) -> 8 cores. Each core projects Q for its
1024 query rows, projects the full new K/V for its batch (duplicated across the
core pair), and runs softmax(Q K^T / 8) V over the 8192-row concatenated cache.

Layout strategy (everything kept in matmul-native layouts, no on-device
transposes):
  - scores are computed TRANSPOSED: S^T[t, s] with lhsT = K^T tile, rhs = Q^T.
  - softmax over t (partition dim) uses exp with a constant shift (exact:
    softmax is shift-invariant) and gets the denominator from an extra
    all-ones column appended to V, so P^T @ [V | 1] yields both the
    numerator rows and the denominator in one PSUM accumulation.
  - P^T is exactly the stationary operand layout the PV matmul needs, so no
    transposes are ever required.
All matmuls run in float32r (full-rate fp32 mode, ~1.5e-4 rms error).
"""
import sys
import numpy as np

if "/opt/trn_rl_repo" not in sys.path:
    sys.path.insert(0, "/opt/trn_rl_repo")

import concourse.bacc as bacc
import concourse.mybir as mybir
from concourse.tile import TileContext
from concourse.bass_utils import run_bass_kernel_spmd

B, S_NEW, S_CACHE, D = 4, 2048, 6144, 1024
S_KV = S_CACHE + S_NEW            # 8192
SQ = S_NEW // 2                   # 1024 query rows per core
N_CORES = 8
P = 128
ET = D // P                       # 8 feature tiles
DT = D // P                       # 8 contraction tiles
CHUNK = 512                       # kv rows per chunk
N_CHUNKS = S_KV // CHUNK          # 16 (12 cached + 4 new)
N_CACHED_CHUNKS = S_CACHE // CHUNK
TT4 = CHUNK // P                  # 4 t-ptiles per chunk
SCALE = 0.125                     # 1/sqrt(64)
SHIFT = -16.0                     # constant softmax shift (exact)

F32 = mybir.dt.float32
F32R = mybir.dt.float32r

_cache = {}


def _build():
    nc = bacc.Bacc("TRN2", target_bir_lowering=False, debug=False,
                   num_devices=N_CORES)
    ht = nc.dram_tensor("ht", [P, DT * S_NEW], F32R, kind="ExternalInput")
    wq = nc.dram_tensor("wq", [P, DT * D], F32R, kind="ExternalInput")
    wk = nc.dram_tensor("wk", [P, DT * D], F32R, kind="ExternalInput")
    wv = nc.dram_tensor("wv", [P, DT * D], F32R, kind="ExternalInput")
    kcT = nc.dram_tensor("kcT", [P, ET, S_CACHE], F32R, kind="ExternalInput")
    vc = nc.dram_tensor("vc", [P, S_CACHE // P, D], F32R, kind="ExternalInput")
    bq = nc.dram_tensor("bq", [P, ET], F32, kind="ExternalInput")
    bk = nc.dram_tensor("bk", [P, ET], F32, kind="ExternalInput")
    bv = nc.dram_tensor("bv", [P, D], F32, kind="ExternalInput")
    ident = nc.dram_tensor("ident", [P, P], F32, kind="ExternalInput")
    out = nc.dram_tensor("out", [SQ, D], F32, kind="ExternalOutput")

    # Which query half this core handles is baked in host-side: the host
    # rotates ht's t-columns so THIS core's 1024 query rows sit at columns
    # [0, SQ) of every d-tile. The rotation also permutes the new K/V row
    # order identically on both cores of a pair, which is harmless -
    # attention is permutation-invariant over kv rows.

    with TileContext(nc) as tc:
        with tc.tile_pool(name="big", bufs=1) as big, \
             tc.tile_pool(name="bias", bufs=1) as biasp, \
             tc.tile_pool(name="spsum", bufs=2, space="PSUM") as spsum, \
             tc.tile_pool(name="dpsum", bufs=2, space="PSUM") as dpsum, \
             tc.tile_pool(name="opsum", bufs=2, space="PSUM") as opsum, \
             tc.tile_pool(name="dram", bufs=1, space="DRAM") as dpool:

            nkT_d = dpool.tile([P, ET, S_NEW], F32R, name="nkT_d")
            nv_d = dpool.tile([P, S_NEW // P, D], F32R, name="nv_d")

            qT_sb = big.tile([P, ET * SQ], F32R, name="qT_sb")
            bq_sb = biasp.tile([P, ET], F32, name="bq_sb")
            bk_sb = biasp.tile([P, ET], F32, name="bk_sb")
            bv_sb = biasp.tile([P, D], F32, name="bv_sb")
            sh_sb = biasp.tile([P, 1], F32, name="sh_sb")
            nc.vector.memset(sh_sb[:], SHIFT)
            ones_sb = biasp.tile([P, 2], F32, name="ones_sb")
            nc.vector.memset(ones_sb[:], 1.0)
            onesr_sb = biasp.tile([P, 2], F32R, name="onesr_sb")
            nc.vector.tensor_copy(onesr_sb[:], ones_sb[:])
            id_sb = biasp.tile([P, P], F32, name="id_sb")
            nc.sync.dma_start(out=id_sb[:], in_=ident[:])

            nc.sync.dma_start(out=bq_sb[:], in_=bq[:])
            nc.sync.dma_start(out=bk_sb[:], in_=bk[:])
            nc.sync.dma_start(out=bv_sb[:], in_=bv[:])

            early_cm = tc.tile_pool(name="early", bufs=1)
            earlyp = early_cm.__enter__()
            kt0_sb = earlyp.tile([P, ET, CHUNK], F32R, name="kt0_sb")
            v0_sb = earlyp.tile([P, TT4, D], F32R, name="v0_sb")

            self_a = tc.tile_pool(name="abig", bufs=1)
            abig = self_a.__enter__()
            wpool_cm = tc.tile_pool(name="wpool", bufs=2)
            wpool = wpool_cm.__enter__()
            stage_cm = tc.tile_pool(name="stage", bufs=3)
            stagep = stage_cm.__enter__()

            wq_sb = wpool.tile([P, DT * D], F32R, name="w_sb", tag="w")
            ht_sb = abig.tile([P, DT * S_NEW], F32R, name="ht_sb")
            for dt in range(DT):
                nc.sync.dma_start(out=wq_sb[:, dt * D:(dt + 1) * D],
                                  in_=wq[:, dt * D:(dt + 1) * D])
                nc.sync.dma_start(out=ht_sb[:, dt * S_NEW:(dt + 1) * S_NEW],
                                  in_=ht[:, dt * S_NEW:(dt + 1) * S_NEW])
            wk_sb = wpool.tile([P, DT * D], F32R, name="w_sb2", tag="w")
            nc.sync.dma_start(out=wk_sb[:], in_=wk[:])
            wv_sb = wpool.tile([P, DT * D], F32R, name="w_sb3", tag="w")
            nc.sync.dma_start(out=wv_sb[:], in_=wv[:])
            nc.sync.dma_start(out=kt0_sb[:], in_=kcT[:, :, 0:CHUNK])
            nc.sync.dma_start(out=v0_sb[:], in_=vc[:, 0:TT4, :])

            # ---- Phase A1: Q^T projection (queries are ht cols [0, SQ)) ----
            for et in range(ET):
                for sc in range(SQ // 512):
                    ps = spsum.tile([P, 512], F32, name="ps_q", tag="sp")
                    for dt in range(DT):
                        nc.tensor.matmul(
                            ps[:],
                            wq_sb[:, dt * D + et * P:dt * D + (et + 1) * P],
                            ht_sb[:, dt * S_NEW + sc * 512:dt * S_NEW + (sc + 1) * 512],
                            start=(dt == 0), stop=(dt == DT - 1))
                    nc.scalar.activation(
                        qT_sb[:, et * SQ + sc * 512:et * SQ + (sc + 1) * 512],
                        ps[:], mybir.ActivationFunctionType.Identity,
                        bias=bq_sb[:, et:et + 1])

            # ---- Phase A2: new K^T -> DRAM scratch ----
            for et in range(ET):
                for sc in range(S_NEW // 512):
                    ps = spsum.tile([P, 512], F32, name="ps_k", tag="sp")
                    for dt in range(DT):
                        nc.tensor.matmul(
                            ps[:],
                            wk_sb[:, dt * D + et * P:dt * D + (et + 1) * P],
                            ht_sb[:, dt * S_NEW + sc * 512:dt * S_NEW + (sc + 1) * 512],
                            start=(dt == 0), stop=(dt == DT - 1))
                    st = stagep.tile([P, 512], F32R, name="st_k", tag="stage")
                    nc.scalar.activation(
                        st[:], ps[:], mybir.ActivationFunctionType.Identity,
                        bias=bk_sb[:, et:et + 1])
                    nc.scalar.dma_start(
                        out=nkT_d[:, et, sc * 512:(sc + 1) * 512], in_=st[:])

            # ---- Phase A3: new V -> DRAM scratch ----
            for tt in range(S_NEW // P):
                for ec in range(D // 512):
                    ps = spsum.tile([P, 512], F32, name="ps_v", tag="sp")
                    for dt in range(DT):
                        nc.tensor.matmul(
                            ps[:],
                            ht_sb[:, dt * S_NEW + tt * P:dt * S_NEW + (tt + 1) * P],
                            wv_sb[:, dt * D + ec * 512:dt * D + (ec + 1) * 512],
                            start=(dt == 0), stop=(dt == DT - 1))
                    st = stagep.tile([P, 512], F32R, name="st_v", tag="stage")
                    nc.vector.tensor_add(st[:], ps[:], bv_sb[:, ec * 512:(ec + 1) * 512])
                    nc.scalar.dma_start(out=nv_d[:, tt, ec * 512:(ec + 1) * 512],
                                        in_=st[:])

            # free phase-A SBUF (ht, weights, staging) for phase-B pools
            stage_cm.__exit__(None, None, None)
            wpool_cm.__exit__(None, None, None)
            self_a.__exit__(None, None, None)

            kpool_cm = tc.tile_pool(name="kpool", bufs=2)
            kpool = kpool_cm.__enter__()
            vpool_cm = tc.tile_pool(name="vpool", bufs=2)
            vpool = vpool_cm.__enter__()
            ptpool_cm = tc.tile_pool(name="ptpool", bufs=2)
            ptpool = ptpool_cm.__enter__()
            fin_cm = tc.tile_pool(name="fin", bufs=2)
            finp = fin_cm.__enter__()
            obig_cm = tc.tile_pool(name="obig", bufs=1)
            obig = obig_cm.__enter__()
            out_acc = obig.tile([P, SQ // P, D], F32, name="out_acc")
            dn_acc = obig.tile([2, SQ], F32, name="dn_acc")

            # ---- Phase B: attention over 16 kv chunks ----
            for c in range(N_CHUNKS):
                if c == 0:
                    kt_sb, v_sb = kt0_sb, v0_sb
                elif True:
                    kt_sb = kpool.tile([P, ET, CHUNK], F32R, name="kt_sb")
                    v_sb = vpool.tile([P, TT4, D], F32R, name="v_sb")
                if c == 0:
                    pass
                elif c < N_CACHED_CHUNKS:
                    nc.sync.dma_start(out=kt_sb[:],
                                      in_=kcT[:, :, c * CHUNK:(c + 1) * CHUNK])
                    nc.sync.dma_start(out=v_sb[:],
                                      in_=vc[:, c * TT4:(c + 1) * TT4, :])
                else:
                    cc = c - N_CACHED_CHUNKS
                    nc.sync.dma_start(out=kt_sb[:],
                                      in_=nkT_d[:, :, cc * CHUNK:(cc + 1) * CHUNK])
                    nc.sync.dma_start(out=v_sb[:],
                                      in_=nv_d[:, cc * TT4:(cc + 1) * TT4, :])

                for sb in range(SQ // 512):
                    pt = ptpool.tile([P, TT4, 512], F32R, name="pt")
                    for tt4 in range(TT4):
                        stp = spsum.tile([P, 512], F32, name="stp", tag="sp")
                        for et in range(ET):
                            nc.tensor.matmul(
                                stp[:],
                                kt_sb[:, et, tt4 * P:(tt4 + 1) * P],
                                qT_sb[:, et * SQ + sb * 512:et * SQ + (sb + 1) * 512],
                                start=(et == 0), stop=(et == ET - 1))
                        nc.scalar.activation(
                            pt[:, tt4, :], stp[:],
                            mybir.ActivationFunctionType.Exp,
                            bias=sh_sb[:], scale=SCALE)
                    dps = dpsum.tile([2, 512], F32, name="dps", tag="dps")
                    for tt4 in range(TT4):
                        nc.tensor.matmul(dps[:], onesr_sb[:],
                                         pt[:, tt4, :],
                                         start=(tt4 == 0), stop=(tt4 == TT4 - 1))
                    if c == 0:
                        nc.vector.tensor_copy(
                            dn_acc[0:2, sb * 512:(sb + 1) * 512], dps[0:2, :])
                    else:
                        nc.vector.tensor_add(
                            dn_acc[0:2, sb * 512:(sb + 1) * 512],
                            dn_acc[0:2, sb * 512:(sb + 1) * 512], dps[0:2, :])
                    for si in range(4):
                        si_g = sb * 4 + si
                        po = opsum.tile([P, D], F32, name="po")
                        for tt4 in range(TT4):
                            lhs = pt[:, tt4, si * P:(si + 1) * P]
                            st0 = (tt4 == 0)
                            sp1 = (tt4 == TT4 - 1)
                            nc.tensor.matmul(po[:, 0:512], lhs,
                                             v_sb[:, tt4, 0:512],
                                             start=st0, stop=sp1)
                            nc.tensor.matmul(po[:, 512:1024], lhs,
                                             v_sb[:, tt4, 512:1024],
                                             start=st0, stop=sp1)
                        if c == 0:
                            nc.vector.tensor_copy(out_acc[:, si_g, :], po[:])
                        else:
                            nc.vector.tensor_add(out_acc[:, si_g, :],
                                                 out_acc[:, si_g, :], po[:])
                        if c == N_CHUNKS - 1:
                            tps = dpsum.tile([P, 2], F32, name="tps", tag="dps")
                            nc.tensor.matmul(
                                tps[:], dn_acc[0:2, si_g * P:(si_g + 1) * P],
                                id_sb[0:2, 0:2], start=True, stop=True)
                            rec = finp.tile([P, 1], F32, name="rec")
                            nc.vector.reciprocal(rec[:], tps[:, 0:1])
                            ost = finp.tile([P, D], F32, name="ost")
                            nc.scalar.activation(
                                ost[:], out_acc[:, si_g, :D],
                                mybir.ActivationFunctionType.Copy,
                                scale=rec[:])
                            nc.sync.dma_start(
                                out=out[si_g * P:(si_g + 1) * P, :], in_=ost[:])

            obig_cm.__exit__(None, None, None)
            fin_cm.__exit__(None, None, None)
            ptpool_cm.__exit__(None, None, None)
            vpool_cm.__exit__(None, None, None)
            kpool_cm.__exit__(None, None, None)
            early_cm.__exit__(None, None, None)

    nc.compile()
    return nc


def _prep(hidden_states, cached_key, cached_value, Wq, bq, Wk, bk, Wv, bv):
    """Host-side resharding into SBUF-image layouts (pure reshapes/copies)."""
    def ptile_cols(a):  # [R, C] with R = n*128 -> [128, n*C] (partition-major)
        n = a.shape[0] // P
        return np.ascontiguousarray(
            a.reshape(n, P, a.shape[1]).transpose(1, 0, 2)).reshape(P, -1)

    w_h = {}
    for nm, W in (("wq", Wq), ("wk", Wk), ("wv", Wv)):
        w_h[nm] = ptile_cols(np.ascontiguousarray(W.T))          # [128, 8*1024]
    bq_h = np.ascontiguousarray(bq.reshape(ET, P).T)             # [128, 8]
    bk_h = np.ascontiguousarray(bk.reshape(ET, P).T)
    bv_h = np.ascontiguousarray(np.broadcast_to(bv, (P, D)))     # [128, 1024]
    id_h = np.eye(P, dtype=np.float32)

    in_maps = []
    for b in range(B):
        ht_full = ptile_cols(np.ascontiguousarray(hidden_states[b].T))  # [128, 8*2048]
        kcT_h = ptile_cols(np.ascontiguousarray(cached_key[b].T)) \
            .reshape(P, ET, S_CACHE)
        vc_h = np.ascontiguousarray(
            cached_value[b].reshape(S_CACHE // P, P, D).transpose(1, 0, 2))
        for h in range(2):
            # pack this core's query rows into ht cols [0, SQ) of each d-tile
            ht_v = ht_full.reshape(P, DT, S_NEW)
            if h == 0:
                ht_c = ht_full
            else:
                ht_c = np.ascontiguousarray(
                    np.concatenate([ht_v[:, :, SQ:], ht_v[:, :, :SQ]], axis=2)
                ).reshape(P, DT * S_NEW)
            in_maps.append({
                "ht": ht_c, "kcT": kcT_h, "vc": vc_h,
                "wq": w_h["wq"], "wk": w_h["wk"], "wv": w_h["wv"],
                "bq": bq_h, "bk": bk_h, "bv": bv_h, "ident": id_h,
                "hsel": np.zeros((1, 1), np.float32),
            })
    return in_maps


def kernel(hidden_states, cached_key, cached_value, Wq, bq, Wk, bk, Wv, bv,
           _trace=False):
    if "nc" not in _cache:
        _cache["nc"] = _build()
    nc = _cache["nc"]
    in_maps = _prep(
        np.asarray(hidden_states, dtype=np.float32),
        np.asarray(cached_key, dtype=np.float32),
        np.asarray(cached_value, dtype=np.float32),
        np.asarray(Wq, dtype=np.float32), np.asarray(bq, dtype=np.float32),
        np.asarray(Wk, dtype=np.float32), np.asarray(bk, dtype=np.float32),
        np.asarray(Wv, dtype=np.float32), np.asarray(bv, dtype=np.float32))
    res = run_bass_kernel_spmd(nc, in_maps, list(range(N_CORES)), trace=_trace)
    _cache["last_result"] = res
    out = np.empty((B, S_NEW, D), np.float32)
    for b in range(B):
        for h in range(2):
            out[b, h * SQ:(h + 1) * SQ, :] = res.results[2 * b + h]["out"]
    return out
